# revision 1
# baseline (speedup 1.0000x reference)
"""Trainium2 Bass kernel for nn_Block_50130858279051 (dense transformer block).

Sharding: 8 cores = 2 batch groups x 4-way tensor parallel.
  - Within a group of 4 cores (one batch): each core computes LN1 (duplicated),
    QKV and attention for its 4 heads over all 2048 tokens; an 8-rank
    AllToAll ships each head's Y to the token owner (cross-batch recv blocks
    are neutralized by zero rows in the per-core proj weights); each core then
    does full proj + residual + LN2 + MLP + residual for its 512-token chunk.
  - Host concatenates the 8 chunks into the full [2, 2048, 1024] output.

All matmuls bf16 with fp32 PSUM accumulation. LN gains/biases and the
1/sqrt(hd) attention scale are folded into weights on the host (biases in
this problem are all zero and are skipped on device). LN stats run on DVE
(bn_stats/bn_aggr); normalization is fused into the PE transpose via a
diag(rstd) rhs. LN1/qkv/attention are emitted interleaved per 512-token
slice so the per-engine in-order streams pipeline. Softmax runs without
max-subtraction (logits are O(5)); scores are exp'd in [P,1024] pairs with
the AV matmul software-pipelined one pair behind; denominators come from an
appended ones-column on V; 1/l is broadcast across the 64 head partitions
with a gpsimd partition_broadcast. The head-pair A2As fire as soon as each
pair's attention drains, overlapping the collective with remaining work,
and proj accumulates pair 0's channels first.
"""
import sys

sys.path.insert(0, "/opt/trn_rl_repo")

import numpy as np
import ml_dtypes
from contextlib import ExitStack

import concourse.bacc as bacc
import concourse.mybir as mybir
import concourse.tile as tile
from concourse.bass_utils import run_bass_kernel_spmd

B, T, C, H, HD = 2, 2048, 1024, 16, 64
HID = 4 * C
P = 128
NCORES, TPG = 8, 4          # 2 groups x 4 cores
TCHUNK = T // TPG           # 512 tokens per core in the MLP phase
HPC = H // TPG              # 4 heads per core
CS = C // P                 # 8 channel subtiles
TT = T // P                 # 16 token tiles
NQ = T // 512               # 4 query slices of 512
NT2 = TCHUNK // P           # 4 token tiles in the chunk
NS = HID // P               # 32 hidden subtiles
f32, bf16, f16 = mybir.dt.float32, mybir.dt.bfloat16, mybir.dt.float16
BF = ml_dtypes.bfloat16
ACT = mybir.ActivationFunctionType

DEBUG = False


def build_nc(debug=False, sim_mode=False, do_p1=True, do_p2=True,
             skip_proj=False, skip_ln2=False, skip_fc=False,
             skip_fc2=False):
    nc = bacc.Bacc("TRN2", target_bir_lowering=False, debug=False,
                   num_devices=NCORES, num_swdge_queues=4)
    xb = nc.declare_dram_parameter("xb", [T, C], bf16, isOutput=False)
    xc = nc.declare_dram_parameter("xc", [TCHUNK, C], bf16, isOutput=False)
    wq = nc.declare_dram_parameter("wq", [P, CS, 2 * P], bf16, isOutput=False)
    wk = nc.declare_dram_parameter("wk", [P, CS, 2 * P], bf16, isOutput=False)
    wv = nc.declare_dram_parameter("wv", [P, CS, 2 * P], bf16, isOutput=False)
    pw = nc.declare_dram_parameter("pw", [P, 2 * CS, C], bf16, isOutput=False)
    fw = nc.declare_dram_parameter("fw", [8, P, CS, 512], bf16, isOutput=False)
    f2w = nc.declare_dram_parameter("f2w", [16, P, 4, 512], bf16,
                                isOutput=False)
    consts = nc.declare_dram_parameter("consts", [P, P + 4 * 512], bf16,
                                       isOutput=False)
    out = nc.declare_dram_parameter("out", [TCHUNK, C], bf16, isOutput=True)
    dbg = {}
    if debug:
        dbg["xlt"] = nc.declare_dram_parameter("dbg_xlt", [P, CS, T], bf16,
                                               isOutput=True)
        dbg["q0"] = nc.declare_dram_parameter("dbg_q0", [P, T], bf16,
                                              isOutput=True)
        dbg["k0"] = nc.declare_dram_parameter("dbg_k0", [P, T], bf16,
                                              isOutput=True)
        dbg["v"] = nc.declare_dram_parameter("dbg_v", [P, TT, HPC, 65], bf16,
                                             isOutput=True)
        dbg["y"] = nc.declare_dram_parameter("dbg_y", [64, HPC, T], bf16,
                                             isOutput=True)
        dbg["z"] = nc.declare_dram_parameter("dbg_z", [T, C], f16,
                                             isOutput=True)
        dbg["x2"] = nc.declare_dram_parameter("dbg_x2", [P, NT2, C], f32,
                                              isOutput=True)
        dbg["ht"] = nc.declare_dram_parameter("dbg_ht", [P, NS, TCHUNK], bf16,
                                              isOutput=True)

    with tile.TileContext(nc) as tc, ExitStack() as ctx:
        per = ctx.enter_context(tc.tile_pool(name="persist", bufs=1))
        wpool = ctx.enter_context(tc.tile_pool(name="wpool", bufs=1))
        work = ctx.enter_context(tc.tile_pool(name="work", bufs=3))
        small = ctx.enter_context(tc.tile_pool(name="small", bufs=3))
        psmm = ctx.enter_context(
            tc.tile_pool(name="psmm", bufs=2, space="PSUM"))
        dram = ctx.enter_context(tc.tile_pool(name="dram", bufs=1,
                                              space="DRAM"))

        # ---- constants -------------------------------------------------
        # ident + causal masks come precomputed from the host so the only
        # Pool ucode op is partition_broadcast (one Q7 library load).
        cst = per.tile([P, P + 4 * 512], bf16, tag="cst")
        nc.gpsimd.dma_start(cst[:], consts[:])
        ident = cst[:, 0:P]
        masks = cst[:, P:].rearrange("p (j t) -> p j t", j=4)
        eps_t = per.tile([P, 1], f32, tag="eps")
        nc.vector.memset(eps_t[:], 1e-5)


        # weight stream pools opened early so the first chunks prefetch
        # during phase 1; chunks rotate across SWDGE/HWDGE queues.
        fws = ctx.enter_context(tc.tile_pool(name="fwstream", bufs=2))
        f2s = ctx.enter_context(tc.tile_pool(name="f2stream", bufs=2))
        ypool = ctx.enter_context(tc.tile_pool(name="ypool", bufs=1))
        # DMA issue engines: keep ACT/DVE free (dma_start costs the issuing
        # sequencer ~670ns except on Pool's SWDGE ring)
        engs = [nc.gpsimd, nc.gpsimd, nc.sync, nc.sync]
        # chunk tiles allocated now; the first prefetch DMAs are issued at
        # q-slice 1 (see below) so they don't jam the startup queues ahead
        # of consts/xb/wq/wk/wv.
        fw_chunks = [fws.tile([P, CS, 512], bf16, tag="fwch", name=f"fwch{g}")
                     for g in range(8)]
        f2_chunks = [f2s.tile([P, 4, 512], bf16, tag="f2ch", name=f"f2ch{i}")
                     for i in range(16)]

        # phase-2 inputs with no phase-1 dependencies, prefetched at ts==1;
        # recv tiles filled inside phase 1 right after each A2A so the
        # first collective's payload lands while attention still runs.
        ylt = [ypool.tile([P, CS, TCHUNK], bf16, tag=f"ylt{pr}",
                          name=f"ylt{pr}") for pr in range(2)]
        ct = ypool.tile([P, NT2, C], bf16, tag="xct")
        pw_s = ypool.tile([P, 2 * CS, C], bf16, tag="pw")

        def issue_prefetch():
            for g in range(4):
                engs[g % 4].dma_start(fw_chunks[g][:], fw[g])
            for idx in range(3):
                engs[idx % 4].dma_start(f2_chunks[idx][:], f2w[idx])
            nc.sync.dma_start(
                ct[:], xc[:].rearrange("(i p) c -> p i c", p=P))
            # proj weights: even (pair-0) subtiles first, they are the
            # first consumed by the proj accumulation order
            pwr_d = pw[:].rearrange("p (j two) c -> p two j c", two=2)
            pwr_s = pw_s[:].rearrange("p (j two) c -> p two j c", two=2)
            nc.gpsimd.dma_start(pwr_s[:, 0], pwr_d[:, 0])
            nc.sync.dma_start(pwr_s[:, 1], pwr_d[:, 1])

        # Two 8-rank AllToAlls, one per head pair (4-rank mesh unsupported):
        # send-block c of A2A[pr] carries my pair-pr heads' Y for token chunk
        # (c %% 4); recv-block j = core j's pair-pr heads for my chunk.
        # Cross-batch blocks are neutralized by zero rows in pw. Pair 0's
        # A2A fires while pair 1's attention for the last slice still runs.
        ybounce = [dram.tile([NCORES * P, TCHUNK], bf16, name=f"ybounce{i}",
                             tag=f"ybounce{i}") for i in range(2)]
        a2a_out = [dram.tile([NCORES * P, TCHUNK], bf16, name=f"a2a_out{i}",
                             tag=f"a2a_out{i}") for i in range(2)]

        def layernorm_transpose(get_tile, n_tiles, dst, i0=0):
            """Token-major [P, C] tiles -> feature-major bf16 dst.

            dst[:, c, i*P:(i+1)*P] = ((x - mu) * rstd)^T for token tile i.
            Stats via DVE bn_stats/bn_aggr; centered copy on ACT; the two
            PSUM evictions split DVE/ACT so neither engine serializes.
            """
            for i in range(i0, i0 + n_tiles):
                xt = get_tile(i)
                stats = small.tile([P, 2, 6], f32, tag="s_bn")
                mv = small.tile([P, 2], f32, tag="s_mv")
                std = small.tile([P, 1], f32, tag="s_std")
                rstd = small.tile([P, 1], f32, tag="s_rstd")
                negmu = small.tile([P, 1], f32, tag="s_negmu")
                cen = work.tile([P, C], bf16, tag="cen")
                for g in range(2):
                    nc.vector.bn_stats(stats[:, g, :],
                                       xt[:, g * 512:(g + 1) * 512])
                nc.vector.bn_aggr(mv[:],
                                  stats[:].rearrange("p g s -> p (g s)"))
                nc.scalar.activation(std[:], mv[:, 1:2], ACT.Sqrt,
                                     bias=eps_t[:])
                nc.vector.reciprocal(rstd[:], std[:])
                nc.vector.tensor_scalar_mul(negmu[:], mv[:, 0:1], -1.0)
                # centered bf16 copy: cen = x - mu  (ACT)
                nc.scalar.activation(cen[:], xt[:], ACT.Identity,
                                     bias=negmu[:])
                dmat = work.tile([P, P], bf16, tag="dmat")
                nc.vector.tensor_scalar_mul(dmat[:], ident, rstd[:])
                for half in range(2):
                    ps = psmm.tile([P, 512], f32, tag="mm")
                    for cq in range(4):
                        c = half * 4 + cq
                        nc.tensor.matmul(
                            ps[:, cq * P:(cq + 1) * P],
                            lhsT=cen[:, c * P:(c + 1) * P],
                            rhs=dmat[:], start=True, stop=True)
                    eng = nc.vector if half == 0 else nc.scalar
                    if eng is nc.vector:
                        eng.tensor_copy(
                            out=dst[:, half * 4:(half + 1) * 4,
                                    i * P:(i + 1) * P],
                            in_=ps[:].rearrange("p (c t) -> p c t", c=4))
                    else:
                        eng.copy(
                            dst[:, half * 4:(half + 1) * 4,
                                i * P:(i + 1) * P],
                            ps[:].rearrange("p (c t) -> p c t", c=4))

        # ================= phase 1: LN1, QKV, attention, proj ===========
        with tc.tile_pool(name="xlt_pool", bufs=1) as xlt_pool, \
             tc.tile_pool(name="at_pool", bufs=3) as at_pool, \
             tc.tile_pool(name="xb_pool", bufs=2) as xb_pool, \
             tc.tile_pool(name="psy", bufs=2, space="PSUM") as psy, \
             tc.tile_pool(name="psatt", bufs=2, space="PSUM") as psatt:
            xlt = xlt_pool.tile([P, CS, T], bf16, tag="xlt")
            vA = xlt_pool.tile([P, TT, HPC, 65], bf16, tag="vA")
            nc.vector.memset(vA[:, :, :, 64:65], 1.0)
            qT = [xlt_pool.tile([P, T], bf16, tag=f"qT{p}", name=f"qT{p}")
                  for p in range(2)]
            kT = [xlt_pool.tile([P, T], bf16, tag=f"kT{p}", name=f"kT{p}")
                  for p in range(2)]
            ysb = [xlt_pool.tile([P, T], bf16, tag=f"ysb{p}",
                                 name=f"ysb{p}") for p in range(2)]
            wq_s = xlt_pool.tile([P, CS, 2 * P], bf16, tag="wq")
            wk_s = xlt_pool.tile([P, CS, 2 * P], bf16, tag="wk")
            wv_s = xlt_pool.tile([P, CS, 2 * P], bf16, tag="wv")
            nc.gpsimd.dma_start(wq_s[:], wq[:])
            nc.gpsimd.dma_start(wk_s[:], wk[:])
            nc.gpsimd.dma_start(wv_s[:], wv[:])

            xb_blocks = {}

            def xb_tile(i):
                blk = i // 4
                if blk not in xb_blocks:
                    t = xb_pool.tile([P, 4, C], bf16, tag="xbblk")
                    src = xb[blk * 512:(blk + 1) * 512, :].rearrange(
                        "(i p) c -> p i c", p=P)
                    eng = nc.sync if blk % 2 == 0 else nc.gpsimd
                    if blk == 0:
                        # tile 0 alone first so the very first LN chain
                        # starts after 256KB instead of 1MB of DMA
                        eng.dma_start(t[:, 0:1, :], src[:, 0:1, :])
                        eng.dma_start(t[:, 1:4, :], src[:, 1:4, :])
                    else:
                        eng.dma_start(t[:], src)
                    xb_blocks[blk] = t
                return xb_blocks[blk][:, i % 4, :]

            # LN1, qkv and attention interleaved per 512-token slice:
            # engines run their streams in order, so LN for slice ts+1 is
            # emitted after attention for slice ts to pipeline DVE/ACT LN
            # work under the PE-heavy qkv+attention of the previous slice.
            def qkv_slice(ts):
                for pair in range(2):
                    for dst_t, wsb in ((qT[pair], wq_s), (kT[pair], wk_s)):
                        ps = psmm.tile([P, 512], f32, tag="mm")
                        for s in range(CS):
                            nc.tensor.matmul(
                                ps[:],
                                lhsT=wsb[:, s, pair * P:(pair + 1) * P],
                                rhs=xlt[:, s, ts * 512:(ts + 1) * 512],
                                start=(s == 0), stop=(s == CS - 1))
                        nc.vector.tensor_copy(
                            out=dst_t[:, ts * 512:(ts + 1) * 512],
                            in_=ps[:])
                for ti in range(4 * ts, 4 * ts + 4):
                    ps = psmm.tile([P, 512], f32, tag="mm")
                    for s in range(CS):
                        nc.tensor.matmul(
                            ps[:, :2 * P],
                            lhsT=xlt[:, s, ti * P:(ti + 1) * P],
                            rhs=wv_s[:, s, :],
                            start=(s == 0), stop=(s == CS - 1))
                    nc.vector.tensor_copy(
                        out=vA[:, ti, :, 0:64],
                        in_=ps[:, :2 * P].rearrange("p (h d) -> p h d",
                                                    h=HPC))

            # ship Y to token owners: ybounce[pr][j-block, :, :] = my head
            # pair pr for rank j's tokens. Sends fire per (qs, pair) as soon
            # as the normalized slice lands so each A2A input is complete
            # when its pair's attention drains.
            ybr2 = [yb[:].rearrange("(j p) t -> j p t", j=NCORES)
                    for yb in ybounce]

            for ts in range(NQ if do_p1 else 0):
                layernorm_transpose(xb_tile, 4, xlt, i0=4 * ts)
                qkv_slice(ts)
                if ts == 1:
                    issue_prefetch()
                qs = ts
                for h in range(HPC):
                    pair, hp = h // 2, h % 2
                    yps = psy.tile([65, 512], f32, tag="yps")
                    nkt = 4 * qs + 4
                    npair = nkt // 2

                    def av_pair(kp, at):
                        for half in range(2):
                            kt = 2 * kp + half
                            nc.tensor.matmul(
                                yps[:], lhsT=vA[:, kt, h, :],
                                rhs=at[:, half, :],
                                start=(kt == 0), stop=(kt == nkt - 1))

                    # k-tiles in pairs: 2 matmuls -> one [P,1024] Exp.
                    # AV for pair kp-1 is emitted after the scores of pair
                    # kp so PE never waits on the just-issued Exp.
                    prev_at = None
                    for kp in range(npair):
                        sps = psatt.tile([P, 2, 512], f32, tag="satt")
                        for half in range(2):
                            kt = 2 * kp + half
                            nc.tensor.matmul(
                                sps[:, half, :],
                                lhsT=kT[pair][hp * 64:(hp + 1) * 64,
                                              kt * P:(kt + 1) * P],
                                rhs=qT[pair][hp * 64:(hp + 1) * 64,
                                             qs * 512:(qs + 1) * 512],
                                start=True, stop=True)
                        if prev_at is not None:
                            av_pair(kp - 1, prev_at)
                        at = at_pool.tile([P, 2, 512], bf16, tag="at")
                        nc.scalar.activation(at[:], sps[:], ACT.Exp)
                        j0 = 2 * kp - 4 * qs
                        if j0 >= 0:
                            nc.vector.tensor_mul(at[:], at[:],
                                                 masks[:, j0:j0 + 2, :])
                        prev_at = at
                    av_pair(npair - 1, prev_at)
                    # normalize: ysb_h = y * (1/l), l = row 64 of yps,
                    # broadcast across the 64 head-dim partitions via Q7
                    lrec = small.tile([1, 512], f32, tag="lrec")
                    nc.vector.reciprocal(lrec[:], yps[64:65, :])
                    rsb = work.tile([64, 512], f32, tag="rsb")
                    nc.gpsimd.partition_broadcast(rsb[:], lrec[:])
                    nc.vector.tensor_mul(
                        ysb[pair][hp * 64:(hp + 1) * 64,
                                  qs * 512:(qs + 1) * 512],
                        yps[0:64, :], rsb[:])
                    if hp == 1:
                        # both heads of the pair done for this slice: ship
                        for j in (qs, qs + 4):
                            eng = nc.sync if j % 2 == 0 else nc.gpsimd
                            eng.dma_start(
                                ybr2[pair][j, :, :],
                                ysb[pair][:, qs * 512:(qs + 1) * 512])
                    if qs == NQ - 1 and hp == 1:
                        # last slice for this pair: its A2A can fire now,
                        # and its recv lands while remaining work runs
                        if sim_mode:
                            # stand-in with roughly the per-core payload
                            # (1 recv block) so sim timing stays comparable
                            nc.sync.dma_start(a2a_out[pair][0:P, :],
                                              ybounce[pair][0:P, :])
                        else:
                            nc.gpsimd.collective_compute(
                                "AllToAll", mybir.AluOpType.bypass,
                                replica_groups=[list(range(NCORES))],
                                ins=[ybounce[pair][:].opt()],
                                outs=[a2a_out[pair][:].opt()])
                        a2r = a2a_out[pair][:].rearrange(
                            "(j p) t -> p j t", p=P)
                        (nc.sync if pair == 0 else nc.gpsimd).dma_start(
                            ylt[pair][:], a2r)

            if debug:
                nc.sync.dma_start(dbg["xlt"][:], xlt[:])
                nc.sync.dma_start(dbg["q0"][:], qT[0][:])
                nc.sync.dma_start(dbg["k0"][:], kT[0][:])
                nc.sync.dma_start(dbg["v"][:], vA[:])
                for h in range(HPC):
                    nc.sync.dma_start(
                        dbg["y"][:, h, :],
                        ysb[h // 2][(h % 2) * 64:(h % 2) * 64 + 64, :])

        if not do_p2:
            with tc.tile_pool(name="dummy_out", bufs=1) as dpool:
                zt0 = dpool.tile([P, C], bf16, tag="zt0")
                nc.vector.memset(zt0[:], 0.0)
                for i in range(NT2):
                    nc.sync.dma_start(out[i * P:(i + 1) * P, :], zt0[:])
            nc.compile()
            return nc

        # ================= phase 2: residual + LN2 + MLP ================
        with tc.tile_pool(name="mlp_per", bufs=1) as mper, \
             tc.tile_pool(name="psfc2", bufs=4, space="PSUM") as psfc2:

            x2 = mper.tile([P, NT2, C], f32, tag="x2")
            x2lt = mper.tile([P, CS, TCHUNK], bf16, tag="x2lt")
            hT = mper.tile([P, NS, TCHUNK], bf16, tag="hT")

            # proj (zero-padded 2C contraction) + residual; pair-0 channels
            # (even s16, from A2A #1) accumulate first so proj starts while
            # A2A #2 is still in flight. LN2 for tile i is emitted right
            # after its residual add so its DVE/ACT work overlaps the next
            # tile's proj matmuls.
            for i in range(0 if skip_proj else NT2):
                for n in range(2):
                    ps = psmm.tile([P, 512], f32, tag="mm")
                    for idx in range(2 * CS):
                        pr, j = idx // CS, idx % CS
                        nc.tensor.matmul(
                            ps[:],
                            lhsT=ylt[pr][:, j, i * P:(i + 1) * P],
                            rhs=pw_s[:, 2 * j + pr, n * 512:(n + 1) * 512],
                            start=(idx == 0), stop=(idx == 2 * CS - 1))
                    nc.vector.tensor_add(
                        x2[:, i, n * 512:(n + 1) * 512], ps[:],
                        ct[:, i, n * 512:(n + 1) * 512])
                if not skip_ln2 and i >= 1:
                    # LN2 for tile i-1: its PE transposes follow proj(i)'s
                    # matmuls so PE never waits on the centered copy
                    layernorm_transpose(lambda ii: x2[:, ii, :], 1, x2lt,
                                        i0=i - 1)
            if not skip_proj and not skip_ln2:
                layernorm_transpose(lambda ii: x2[:, ii, :], 1, x2lt,
                                    i0=NT2 - 1)

            if skip_proj:
                for i in range(NT2):
                    nc.vector.tensor_copy(out=x2[:, i, :], in_=ct[:, i, :])
                if not skip_ln2:
                    layernorm_transpose(lambda i: x2[:, i, :], NT2, x2lt)
            if skip_ln2:
                nc.vector.memset(x2lt[:], 0.001)

            # fc + gelu -> h^T (feature-major); fw streamed in 8 chunks
            if skip_fc:
                nc.vector.memset(hT[:], 0.001)
            for g in range(0 if skip_fc else 8):
                fwch = fw_chunks[g]
                if g >= 4:
                    engs[g % 4].dma_start(fwch[:], fw[g])
                for mq in range(4):
                    m = g * 4 + mq
                    ps = psmm.tile([P, 512], f32, tag="mm")
                    for s in range(CS):
                        nc.tensor.matmul(
                            ps[:],
                            lhsT=fwch[:, s, mq * P:(mq + 1) * P],
                            rhs=x2lt[:, s, :],
                            start=(s == 0), stop=(s == CS - 1))
                    nc.scalar.activation(hT[:, m, :], ps[:], ACT.Gelu)

            if debug:
                nc.sync.dma_start(dbg["x2"][:], x2[:])
                nc.sync.dma_start(dbg["ht"][:], hT[:])

            # fc2 + final residual (token-major out)
            if skip_fc2:
                for i in range(NT2):
                    ot2 = work.tile([P, C], bf16, tag="xbt")
                    nc.vector.tensor_copy(out=ot2[:], in_=x2[:, i, :])
                    nc.sync.dma_start(out[i * P:(i + 1) * P, :], ot2[:])
            for n in range(0 if skip_fc2 else 2):
                pss = [psfc2.tile([P, 512], f32, tag="fc2", name=f"fc2_{n}_{t}")
                       for t in range(NT2)]
                for sg in range(NS // 4):
                    idx = n * 8 + sg
                    f2ch = f2_chunks[idx]
                    if idx >= 3:
                        engs[idx % 4].dma_start(f2ch[:], f2w[idx])
                    for sq in range(4):
                        s = 4 * sg + sq
                        for ti in range(NT2):
                            nc.tensor.matmul(
                                pss[ti][:],
                                lhsT=hT[:, s, ti * P:(ti + 1) * P],
                                rhs=f2ch[:, sq, :],
                                start=(s == 0), stop=(s == NS - 1))
                outt = [work.tile([P, C], bf16, tag="ztw", name=f"ot_{n}_{t}")
                        for t in range(NT2)]
                for ti in range(NT2):
                    nc.vector.tensor_add(
                        outt[ti][:, n * 512:(n + 1) * 512], pss[ti][:],
                        x2[:, ti, n * 512:(n + 1) * 512])
                    nc.sync.dma_start(
                        out[ti * P:(ti + 1) * P, n * 512:(n + 1) * 512],
                        outt[ti][:, n * 512:(n + 1) * 512])

    nc.compile()
    return nc


def _prep_core_inputs(x, ln1_g, ln1_b, attn_w, attn_b, proj_w, proj_b,
                      ln2_g, ln2_b, fc_w, fc_b, fc2_w, fc2_b):
    """Host-side weight folding + per-core slicing. Returns in_maps list."""
    f = np.float32
    x = np.asarray(x, f)
    aw = np.asarray(ln1_g, f)[:, None] * np.asarray(attn_w, f)
    ab = np.asarray(attn_b, f) + np.asarray(ln1_b, f) @ np.asarray(attn_w, f)
    fwf = np.asarray(ln2_g, f)[:, None] * np.asarray(fc_w, f)
    fbf = np.asarray(fc_b, f) + np.asarray(ln2_b, f) @ np.asarray(fc_w, f)
    assert not np.any(ab) and not np.any(fbf), "nonzero qkv/fc bias unsupported"
    assert not np.any(np.asarray(proj_b, f)) and not np.any(
        np.asarray(fc2_b, f)), "nonzero proj/fc2 bias unsupported"

    qw = aw[:, :C] * f(1.0 / np.sqrt(HD))    # fold softmax scale into Wq
    kw = aw[:, C:2 * C]
    vw = aw[:, 2 * C:]
    pwf = np.asarray(proj_w, f)
    f2wf = np.asarray(fc2_w, f)

    def as_lhst(w):  # [K, N] -> [P, K//P, N]
        return np.ascontiguousarray(
            w.reshape(w.shape[0] // P, P, w.shape[1]).transpose(1, 0, 2)
        ).astype(BF)

    # per-group zero-padded proj weights for the 8-rank A2A recv layout:
    # recv-block j (rows 256j..256j+256) is core j's heads; valid iff core j
    # is in this core's batch group, and then equals proj_w rows for heads
    # 4*(j %% 4)..4*(j %% 4)+4.
    pw_pad = np.zeros((2, 2 * C, C), np.float32)
    for g in range(2):
        for j in range(NCORES):
            if j // TPG == g:
                r = j % TPG
                pw_pad[g, 256 * j:256 * (j + 1), :] = \
                    pwf[256 * r:256 * (r + 1), :]

    fw_l = as_lhst(fwf)            # [128, 8, 4096]
    fw_t = np.ascontiguousarray(
        np.stack([fw_l[:, :, g * 512:(g + 1) * 512] for g in range(8)]))
    f2_l = as_lhst(f2wf)           # [128, 32, 1024]
    f2w_t = np.ascontiguousarray(
        np.stack([f2_l[:, 4 * (i % 8):4 * (i % 8) + 4,
                       (i // 8) * 512:(i // 8 + 1) * 512]
                  for i in range(16)]))

    # host-built constants: [P, P] identity + 4 causal diagonal masks
    # (mask j: keep iff ki - qj + 128*j <= 0 on a [128 k, 512 q] tile)
    ident = np.eye(P, dtype=np.float32)
    ki = np.arange(P)[:, None]
    qj = np.arange(512)[None, :]
    mask_list = [(ki - qj + 128 * j <= 0).astype(np.float32)
                 for j in range(4)]
    consts = np.concatenate([ident] + mask_list, axis=1).astype(BF)

    in_maps = []
    for core in range(NCORES):
        b, r = core // TPG, core % TPG
        cols = slice(256 * r, 256 * r + 256)
        in_maps.append({
            "xb": np.ascontiguousarray(x[b]).astype(BF),
            "xc": np.ascontiguousarray(x[b, TCHUNK * r:TCHUNK * (r + 1)]).astype(BF),
            "wq": as_lhst(qw[:, cols]),
            "wk": as_lhst(kw[:, cols]),
            "wv": as_lhst(vw[:, cols]),
            "pw": as_lhst(pw_pad[b]),
            "fw": fw_t,
            "f2w": f2w_t,
            "consts": consts,
        })
    return in_maps


_built = {}


def run(inputs, trace=False, debug=DEBUG, **spmd_kwargs):
    key = ("dbg" if debug else "rel")
    if key not in _built:
        _built[key] = build_nc(debug=debug)
    nc = _built[key]
    in_maps = _prep_core_inputs(**inputs)
    res = run_bass_kernel_spmd(nc, in_maps, list(range(NCORES)),
                               trace=trace, **spmd_kwargs)
    full = np.empty((B, T, C), np.float32)
    for core in range(NCORES):
        b, r = core // TPG, core % TPG
        full[b, TCHUNK * r:TCHUNK * (r + 1)] = res.results[core]["out"]
    return full, res


def kernel(**inputs):
    full, _ = run(inputs, trace=False, debug=False)
    return full

